# revision 1
# baseline (speedup 1.0000x reference)
"""Trainium2 Bass kernel for nn_AttentionLayer_66949950210666.

Cross-attention layer: q from decoder_hs, k/v from encoder_hs,
16 heads, D=1024, S=2048, B=2, fp32.

Sharding (8 cores): core c = (b, r) with b = c // 4, r = c % 4.
Each core handles batch b and heads [4r, 4r+4) (o-dims [256r, 256r+256)).
Device-side, everything lives in a "transposed world":
  QT[o, s], KT[o, s]  (o on partitions)  and V[s, o] (s on partitions),
so the attention works without any on-chip transposes:
  ST[k, q]   = KTz_h^T-contract-d  (lhsT=KTz[h] zero-padded to K=128 so every
                                    matmul stays in the 128x128 tile mode)
  PT[k, q]   = exp(ST / 8)                       (ScalarE, no max-subtract;
                                                  |S| ~ N(0,1), fp32-safe)
  YuT[d, q]  = sum_k Vaug[k, d|ones] PT[k, q]    (K=128 matmul; the extra
                                                  "ones" column makes row 64
                                                  the softmax denominator)
  YT = YuT * recip(denom)  broadcast across partitions via a tiny K=1 matmul

The kernel is organized as ONE continuous stream: 128 exp ACTIVATEs of
[128,1024] (~142us on ScalarE) paced against a near-saturated PE.  All
projection matmuls (K/V/Q) are injected into the PE stream between
attention iterations, scheduled against their data deadlines (and the
in-order sync-queue DMA arrival times), so the exp stream starts as
early as possible instead of waiting for a serial projection phase.
Pair-1's K/Q projections are deferred into pair-0's iterations.
The softmax normalize uses reciprocal_approx_fast on a [64,512]
broadcast block at base partition 0 (the custom DVE op mishandles
nonzero base partitions), with the raw denominator broadcast via a
row-64-selector K=128 matmul.

Final projection after an AllToAll across the 8 cores: each core
gathers the full Y^T[j=1024, its 256 s-columns] and computes
ZT[o, s_r] = Wp.T-contract-j + c, where c = Wp @ bv + bp is folded on
the host.  Host returns out[b, 256c:256(c+1), :] = ZT.T per core.
"""

import sys

sys.path.insert(0, "/opt/trn_rl_repo")

import ml_dtypes
import numpy as np

import bass_rust as _bass_rust

import concourse.bass as bass
import concourse.mybir as mybir
import concourse.tile as tile
from concourse import bacc
from concourse.bass_utils import run_bass_kernel_spmd

F32 = mybir.dt.float32
BF16 = mybir.dt.bfloat16
F8 = mybir.dt.float8e4

# The greedy ACT-table chooser could ping-pong between table sets; hide
# Exp/Ln from the single-function sets so every activation resolves to
# natural_log_exp_and_others.  Only the membership sets are changed — dict
# order/length (the act_func_set_id space) is untouched.
import concourse.hw_specs as _hw_specs
from concourse import bacc as _bacc_mod

_orig_get_tables = _hw_specs.get_activation_tables


def _patched_get_tables(arch):
    t = {k: set(v) for k, v in _orig_get_tables(arch).items()}
    if "natural_log_exp_and_others" in t:
        for name, fns in t.items():
            if name != "natural_log_exp_and_others":
                fns.discard(mybir.ActivationFunctionType.Exp)
                fns.discard(mybir.ActivationFunctionType.Ln)
    return t


_bacc_mod.get_activation_tables = _patched_get_tables

B, S, D, H, HD = 2, 2048, 1024, 16, 64
NCORES = 8
OL = 256          # local output dims (4 heads x 64)
SB = S // 8       # 256: s-slice per core after the 8-way AllToAll
NST = S // 512    # 4 s-tiles of 512
NDC = D // 128    # 8 contraction chunks
NKT = S // 128    # 16 k-tiles
SCALE = 0.125     # 1/sqrt(HD)


def build_nc():
    nc = bacc.Bacc(None, num_devices=NCORES, target_bir_lowering=False)

    xdT = nc.declare_dram_parameter("xdT", [D, S], BF16, isOutput=False)
    xeT = nc.declare_dram_parameter("xeT", [D, S], BF16, isOutput=False)
    wqT = nc.declare_dram_parameter("wqT", [D, OL], BF16, isOutput=False)
    wkT = nc.declare_dram_parameter("wkT", [D, OL], BF16, isOutput=False)
    wvT = nc.declare_dram_parameter("wvT", [D, OL], BF16, isOutput=False)
    wpT = nc.declare_dram_parameter("wpT", [D, D], BF16, isOutput=False)
    bqP = nc.declare_dram_parameter("bq", [2, 128], F32, isOutput=False)
    bkP = nc.declare_dram_parameter("bk", [2, 128], F32, isOutput=False)
    cbP = nc.declare_dram_parameter("cb", [8, 128], F32, isOutput=False)
    ztO = nc.declare_dram_parameter("zT", [2, D, SB], BF16, isOutput=True)

    with tile.TileContext(nc) as tc:
        with (
            tc.tile_pool(name="const", bufs=1) as const,
            tc.tile_pool(name="big", bufs=1) as big,
            tc.tile_pool(name="xp", bufs=1) as xp,
            tc.tile_pool(name="dram", bufs=1, space="DRAM") as dram,
        ):
            # ---- constants / weights resident in SBUF ----
            # DMA order matters: the sync queue is in-order, so issue in the
            # order the prologue consumes: wk/wv (+tiny biases), xe st0/st1,
            # xd qt0, wq, xe st2/st3.  wq/xd are only needed ~12us in.
            wq_s = const.tile([128, NDC, OL], BF16)
            wk_s = const.tile([128, NDC, OL], BF16)
            wv_s = const.tile([128, NDC, OL], BF16)
            nc.sync.dma_start(wk_s[:], wkT.rearrange("(dc p) o -> p dc o", p=128))
            bq_s = const.tile([128, 2], F32)
            bk_s = const.tile([128, 2], F32)
            cb_s = const.tile([128, 8], F32)
            # row-64 selector: out[m, q] = rhs[64, q] via a K=128 matmul in
            # the standard tile mode (row 64 ones, all other rows zero); db
            # is a persistent, pre-zeroed staging row so the matmul never
            # reads uninitialized SBUF on its zero rows.
            ones_sb = const.tile([128, 128], BF16)
            nc.vector.memset(ones_sb[:], 0.0)
            nc.vector.memset(ones_sb[64:65, :], 1.0)
            db_s = [const.tile([128, 512], BF16, name=f"db{i}") for i in range(2)]
            for i in range(2):
                nc.vector.memset(db_s[i][:], 0.0)

            # persistent activations
            QT = [big.tile([128, S], BF16, tag=f"QT{i}", name=f"QT{i}") for i in range(2)]
            # per-head K tiles, zero-padded on the other head's 64 rows so the
            # scores matmul runs K=128 in the standard 128x128 tile mode (no
            # PE tiling-mode switches against the 128-deep PV/projection
            # matmuls); the zero rows annihilate the other head's Q rows.
            KTz = [big.tile([128, S], BF16, tag=f"KTz{h}", name=f"KTz{h}")
                   for h in range(4)]
            for h in range(4):
                nc.vector.memset(KTz[h][64 * (1 - h % 2):64 * (2 - h % 2), :], 0.0)
            # V augmented with a ones column per head: [k-part, kt, h, 65]
            vaug = big.tile([128, NKT, 4, 65], BF16, tag="vaug")
            nc.vector.memset(vaug[:, :, :, 64:65], 1.0)

            # input staging: one tile + one DMA per 512-s-block (the sync
            # queue issues descriptors serially at ~0.7us each, so fewer,
            # larger DMAs start the pipeline sooner).  Element [p, dch, two,
            # s] holds d-index dch*256 + two*128 + p, so contraction chunk
            # dc lives at [:, dc // 2, dc % 2, :].
            xe_t = [xp.tile([128, 4, 2, 512], BF16, tag="xe", name=f"xe{st}",
                            bufs=4) for st in range(NST)]
            xd_t = [xp.tile([128, 4, 2, 512], BF16, tag="xd", name=f"xd{qt}",
                            bufs=4) for qt in range(NST)]

            def load_xe(st, halves=False):
                if halves:
                    for hf in range(2):
                        nc.sync.dma_start(
                            xe_t[st][:, 2 * hf:2 * hf + 2, :, :],
                            xeT[hf * 512:(hf + 1) * 512,
                                st * 512:(st + 1) * 512]
                            .rearrange("(dch two p) s -> p dch two s",
                                       p=128, two=2))
                        if hf == 0:
                            nc.sync.dma_start(
                                wv_s[:],
                                wvT.rearrange("(dc p) o -> p dc o", p=128))
                else:
                    nc.sync.dma_start(
                        xe_t[st][:],
                        xeT[:, st * 512:(st + 1) * 512]
                        .rearrange("(dch two p) s -> p dch two s", p=128, two=2))

            def load_xd(qt):
                nc.sync.dma_start(
                    xd_t[qt][:],
                    xdT[:, qt * 512:(qt + 1) * 512]
                    .rearrange("(dch two p) s -> p dch two s", p=128, two=2))

            load_xe(0, halves=True)
            nc.sync.dma_start(bk_s[:], bkP.rearrange("a p -> p a"))
            load_xe(1)
            nc.sync.dma_start(bq_s[:], bqP.rearrange("a p -> p a"))
            nc.sync.dma_start(cb_s[:], cbP.rearrange("a p -> p a"))
            load_xd(0)
            nc.sync.dma_start(wq_s[:], wqT.rearrange("(dc p) o -> p dc o", p=128))
            load_xe(2)
            load_xe(3)
            load_xd(1)

            ydramP = [dram.tile([8, 128, SB], BF16, name=f"ydram{p}") for p in range(2)]
            ygathP = [dram.tile([8, 128, SB], BF16, name=f"ygath{p}") for p in range(2)]
            ytgEO = [const.tile([128, 2, NDC // 2, SB], BF16, name=f"ytg{p}")
                     for p in range(2)]

            with (
                tc.tile_pool(name="stp", bufs=2, space="PSUM") as stp,
                tc.tile_pool(name="yup", bufs=2, space="PSUM") as yup,
                tc.tile_pool(name="aux", bufs=2, space="PSUM") as auxp,
                tc.tile_pool(name="pt", bufs=6) as ptp,
                tc.tile_pool(name="ep", bufs=6) as ep,
            ):
                # ---- injected projection groups (each uses one aux slot) ----
                def emit_k(st, oc):
                    ssl = slice(st * 512, (st + 1) * 512)
                    kps = auxp.tile([128, 512], F32, tag="aux", name="kps")
                    for dc in range(NDC):
                        nc.tensor.matmul(
                            kps[:], wk_s[:, dc, oc * 128:(oc + 1) * 128],
                            xe_t[st][:, dc // 2, dc % 2, :],
                            start=(dc == 0), stop=(dc == NDC - 1))
                    # split per head so KTz keeps the other head's rows zero
                    for hh in range(2):
                        psl = slice(64 * hh, 64 * (hh + 1))
                        nc.vector.tensor_scalar_add(
                            KTz[2 * oc + hh][psl, ssl], kps[psl, :],
                            bk_s[psl, oc:oc + 1])

                def emit_v(st, half):
                    # two s-subblocks (kt = 4*st + 2*half + {0,1}) share one
                    # aux slot; one DVE copy moves both into vaug
                    vps = auxp.tile([128, 2, 256], F32, tag="aux", name="vps")
                    for uu in range(2):
                        u = 2 * half + uu
                        for dc in range(NDC):
                            nc.tensor.matmul(
                                vps[:, uu, :],
                                xe_t[st][:, dc // 2, dc % 2,
                                         u * 128:(u + 1) * 128],
                                wv_s[:, dc, :],
                                start=(dc == 0), stop=(dc == NDC - 1))
                    kt0 = 4 * st + 2 * half
                    nc.vector.tensor_copy(
                        vaug[:, kt0:kt0 + 2, :, 0:64],
                        vps[:].rearrange("p u (h d) -> p u h d", h=4))

                def emit_q(qt, oc):
                    qsl = slice(qt * 512, (qt + 1) * 512)
                    qps = auxp.tile([128, 512], F32, tag="aux", name="qps")
                    for dc in range(NDC):
                        nc.tensor.matmul(
                            qps[:], wq_s[:, dc, oc * 128:(oc + 1) * 128],
                            xd_t[qt][:, dc // 2, dc % 2, :],
                            start=(dc == 0), stop=(dc == NDC - 1))
                    nc.vector.tensor_scalar_add(
                        QT[oc][:, qsl], qps[:], bq_s[:, oc:oc + 1])

                def finish_qt(pair, qt, yufs, anchor):
                    # deferred normalize+store; the raw denominator row is
                    # broadcast across partitions via a K=1 matmul (pinned
                    # behind `anchor` so the scheduler cannot hoist it into a
                    # head-of-line block), then the fast approximate
                    # reciprocal runs on the full [64,512] block at base
                    # partition 0 (the custom DVE op mishandles nonzero base
                    # partitions).
                    # phase-ordered across the two heads so the DVE->PE->DVE
                    # semaphore round-trip is paid once per qt, not twice
                    for hh in range(2):
                        nc.vector.tensor_copy(
                            db_s[hh][64:65, :], yufs[hh][64:65, :])
                    rpss = []
                    for hh in range(2):
                        rps = auxp.tile([128, 512], F32, tag="aux", name="rps")
                        rmm = nc.tensor.matmul(
                            rps[:], ones_sb[:, :], db_s[hh][:, :],
                            start=True, stop=True)
                        _bass_rust.add_dep_helper(
                            rmm.ins, anchor.ins, sync=False,
                            reason="pin R-matmul after current attention MMs")
                        rpss.append(rps)
                    ysts = []
                    for hh in range(2):
                        rrec = ep.tile([128, 512], F32, tag="r32", name="rrec")
                        nc.vector.reciprocal_approx_fast(
                            rrec[0:64, :], rpss[hh][0:64, :])
                        yst = ep.tile([64, 512], BF16, tag="yst", name="yst")
                        nc.vector.tensor_mul(
                            yst[:], yufs[hh][0:64, :], rrec[0:64, :])
                        ysts.append(yst)
                    for hh in range(2):
                        nc.sync.dma_start(
                            ydramP[pair][2 * qt:2 * qt + 2,
                                         64 * hh:64 * (hh + 1), :]
                            .rearrange("s j q -> j s q"),
                            ysts[hh][:].rearrange("j (s q) -> j s q", s=2))

                # ---- prologue: minimum work before the exp stream starts ----
                emit_k(0, 0)
                emit_v(0, 0)
                emit_v(0, 1)
                emit_q(0, 0)

                # injection schedule: (pair, qt, kt) -> list of thunks.
                # Deadlines: pair0/qt0 consumes KT[0] st_j at kt=4j and vaug
                # at kt; QT[0] qt at pair0/qt start; KT[1]/QT[1] only at
                # pair1 (iteration 64+), so their projections ride pair-0's
                # PE slack.  Injection points also respect xe/xd DMA arrival
                # times so a projection matmul never head-of-line blocks the
                # PE queue waiting on its input DMA.
                inj = {}

                def at(pair, qt, kt, fn, *a):
                    inj.setdefault((pair, qt, kt), []).append((fn, a))

                at(0, 0, 0, emit_k, 1, 0)
                at(0, 0, 1, emit_v, 1, 0)
                at(0, 0, 2, emit_v, 1, 1)
                at(0, 0, 4, emit_k, 2, 0)
                at(0, 0, 5, emit_v, 2, 0)
                at(0, 0, 6, emit_v, 2, 1)
                at(0, 0, 8, emit_k, 3, 0)
                at(0, 0, 9, emit_v, 3, 0)
                at(0, 0, 10, emit_v, 3, 1)
                at(0, 0, 12, emit_q, 1, 0)
                at(0, 0, 14, emit_q, 0, 1)
                at(0, 1, 2, emit_k, 0, 1)
                at(0, 1, 4, emit_k, 1, 1)
                at(0, 1, 6, load_xd, 2)
                at(0, 1, 10, emit_q, 2, 0)
                at(0, 2, 6, load_xd, 3)
                at(0, 2, 10, emit_q, 3, 0)
                at(1, 0, 2, emit_k, 2, 1)
                at(1, 0, 6, emit_k, 3, 1)
                at(1, 0, 10, emit_q, 1, 1)
                at(1, 1, 2, emit_q, 2, 1)
                at(1, 2, 2, emit_q, 3, 1)

                # prefetch Wp mid-stream (2MB; emitted late enough that it
                # cannot head-of-line block the xd/xe loads above)
                wp_s = const.tile([128, NDC, D], BF16)
                at(0, 3, 2, lambda: nc.sync.dma_start(
                    wp_s[:], wpT.rearrange("(jc p) o -> p jc o", p=128)))

                # ---- the attention stream ----
                # Software-pipelined emission: the scores matmuls for
                # iteration n+1 are emitted BEFORE iteration n's PV matmuls.
                # The in-order PE queue would otherwise serialize
                # exp(n) -> PV(n) -> scores(n+1) -> exp(n+1) every iteration;
                # with scores hoisted one step, PE computes scores(n+1)
                # (which only needs the super freed by exp(n-1)) while the
                # scalar engine runs exp(n).
                def emit_scores(pair, qt, kt):
                    sps = stp.tile([128, 1024], F32, tag="st")
                    for hh in range(2):
                        nc.tensor.matmul(
                            sps[:, 512 * hh:512 * (hh + 1)],
                            KTz[2 * pair + hh][:, kt * 128:(kt + 1) * 128],
                            QT[pair][:, qt * 512:(qt + 1) * 512],
                            start=True, stop=True)
                    return sps

                iters = [(pair, qt, kt) for pair in range(2)
                         for qt in range(NST) for kt in range(NKT)]
                pending = None
                yu = None
                # two supers in flight: scores for n+2 are emitted during
                # iteration n, so they execute inside exp(n)'s window and
                # exp(n+1) is never gated on a fresh scores matmul
                sps_q = [emit_scores(*iters[0]), emit_scores(*iters[1])]
                for idx, (pair, qt, kt) in enumerate(iters):
                    if kt == 0:
                        yu = [yup.tile([128, 512], F32, tag="yu",
                                       name=f"yu{hh}") for hh in range(2)]
                    sps = sps_q.pop(0)
                    pt_t = ptp.tile([128, 1024], BF16, tag="pt")
                    nc.scalar.activation(
                        pt_t[:], sps[:],
                        mybir.ActivationFunctionType.Exp, scale=SCALE)
                    if idx + 2 < len(iters):
                        sps_q.append(emit_scores(*iters[idx + 2]))
                    for hh in range(2):
                        h = 2 * pair + hh
                        last_pv = nc.tensor.matmul(
                            yu[hh][0:65, :], vaug[:, kt, h, :],
                            pt_t[:, 512 * hh:512 * (hh + 1)],
                            start=(kt == 0), stop=(kt == NKT - 1))
                    for fn, a in inj.get((pair, qt, kt), ()):
                        fn(*a)
                    if kt == 6 and pending is not None:
                        finish_qt(pair, *pending, anchor=last_pv)
                        pending = None
                    if kt == NKT - 1:
                        # evacuate PSUM immediately (DVE only, no PE ops)
                        yufs = []
                        for hh in range(2):
                            yuf = ep.tile([65, 512], F32, tag="yuf",
                                          name="yuf")
                            nc.vector.tensor_copy(yuf[:], yu[hh][0:65, :])
                            yufs.append(yuf)
                        pending = (qt, yufs)
                        if qt == NST - 1:
                            finish_qt(pair, *pending, anchor=last_pv)
                            pending = None
                            # pair's AllToAll: pair 0's hides under pair 1's
                            # compute
                            nc.gpsimd.collective_compute(
                                "AllToAll", mybir.AluOpType.bypass,
                                replica_groups=[list(range(NCORES))],
                                ins=[ydramP[pair].opt()],
                                outs=[ygathP[pair].opt()])

                # gathers emitted after all yst DMAs so they cannot
                # head-of-line block the sync queue while waiting on the CC
                for pair in range(2):
                    # ygathP[p][4*bb+g] = Y^T rows for global j chunk 2g+p
                    nc.sync.dma_start(
                        ytgEO[pair][:],
                        ygathP[pair][:].rearrange("(bb g) j q -> j bb g q", bb=2))

            # ---- output projection: pair-0 half runs during the second
            # AllToAll; both batches fused into N=512 matmuls ----
            zacc = const.tile([128, NDC, 512], F32)
            with (
                tc.tile_pool(name="zps", bufs=4, space="PSUM") as zpsp,
                tc.tile_pool(name="zt", bufs=4) as ztp,
            ):
                for oc in range(NDC):
                    zps = zpsp.tile([128, 512], F32, tag="z", name="zps")
                    for g in range(4):
                        nc.tensor.matmul(
                            zps[:], wp_s[:, 2 * g, oc * 128:(oc + 1) * 128],
                            ytgEO[0][:, :, g, :],
                            start=(g == 0), stop=(g == 3))
                    nc.vector.tensor_copy(zacc[:, oc, :], zps[:])
                for oc in range(NDC):
                    zps = zpsp.tile([128, 512], F32, tag="z", name="zps2")
                    for g in range(4):
                        nc.tensor.matmul(
                            zps[:], wp_s[:, 2 * g + 1, oc * 128:(oc + 1) * 128],
                            ytgEO[1][:, :, g, :],
                            start=(g == 0), stop=(g == 3))
                    zt_t = ztp.tile([128, 512], BF16, tag="zt", name="zt_t")
                    nc.vector.scalar_tensor_tensor(
                        zt_t[:], zps[:], cb_s[:, oc:oc + 1],
                        zacc[:, oc, :],
                        op0=mybir.AluOpType.add, op1=mybir.AluOpType.add)
                    nc.sync.dma_start(
                        ztO[:, oc * 128:(oc + 1) * 128, :]
                        .rearrange("b p q -> p b q"),
                        zt_t[:].rearrange("p (b q) -> p b q", b=2))

    nc.compile()
    return nc


def make_in_maps(decoder_hs, encoder_hs, Wq, bq, Wk, bk, Wv, bv, Wp, bp):
    dh = np.ascontiguousarray(np.asarray(decoder_hs, np.float32))
    eh = np.ascontiguousarray(np.asarray(encoder_hs, np.float32))
    Wq, Wk, Wv, Wp = (np.asarray(a, np.float32) for a in (Wq, Wk, Wv, Wp))
    bq, bk, bv, bp = (np.asarray(a, np.float32) for a in (bq, bk, bv, bp))
    c = (Wp @ bv + bp).astype(np.float32)
    bf = ml_dtypes.bfloat16
    wpT = np.ascontiguousarray(Wp.T).astype(bf)
    xdT = [np.ascontiguousarray(dh[b].T).astype(bf) for b in range(B)]
    xeT = [np.ascontiguousarray(eh[b].T).astype(bf) for b in range(B)]
    in_maps = []
    for core in range(NCORES):
        b, r = divmod(core, 4)
        sl = slice(OL * r, OL * (r + 1))
        in_maps.append({
            "xdT": xdT[b],
            "xeT": xeT[b],
            "wqT": np.ascontiguousarray(Wq[sl].T).astype(bf),
            "wkT": np.ascontiguousarray(Wk[sl].T).astype(bf),
            "wvT": np.ascontiguousarray(Wv[sl].T).astype(bf),
            "wpT": wpT,
            "bq": np.ascontiguousarray(bq[sl].reshape(2, 128)),
            "bk": np.ascontiguousarray(bk[sl].reshape(2, 128)),
            "cb": np.ascontiguousarray(c.reshape(8, 128)),
        })
    return in_maps


def assemble_output(results):
    out = np.empty((B, S, D), np.float32)
    for core in range(NCORES):
        zT = np.asarray(results[core]["zT"])  # [2, 1024, 256]
        for b in range(B):
            out[b, SB * core:SB * (core + 1), :] = zT[b].T
    return out


_NC = None


def kernel(**inputs):
    global _NC
    if _NC is None:
        _NC = build_nc()
    in_maps = make_in_maps(**inputs)
    res = run_bass_kernel_spmd(_NC, in_maps, list(range(NCORES)))
    return assemble_output(res.results)


if __name__ == "__main__":
    nc = build_nc()
    print("built ok")



# revision 7
# speedup vs baseline: 1.0502x; 1.0502x over previous
"""Trainium2 Bass kernel for nn_AttentionLayer_66949950210666.

Cross-attention layer: q from decoder_hs, k/v from encoder_hs,
16 heads, D=1024, S=2048, B=2, fp32.

Sharding (8 cores): core c = (b, r) with b = c // 4, r = c % 4.
Each core handles batch b and heads [4r, 4r+4) (o-dims [256r, 256r+256)).
Device-side, everything lives in a "transposed world":
  QT[o, s], KT[o, s]  (o on partitions)  and V[s, o] (s on partitions),
so the attention works without any on-chip transposes.

Scores use ROW-TILED PACKED matmuls: each head's K/Q live on a 64-
partition half (head hh of a pair at partitions [64hh, 64hh+64)), so the
two heads' scores matmuls are K=64 matmuls on DISTINCT PE row-groups
(tile_position (0,0) / (64,0)) and execute CONCURRENTLY — one N=512 slot
for both heads instead of two zero-padded K=128 slots.

  ST[k, q]   = KT2[pair][64hh:64hh+64]^T-contract-d  (row-tiled pair)
  PT[k, q]   = exp(ST / 8)                       (ScalarE, no max-subtract;
                                                  |S| ~ N(0,1), fp32-safe)
  YuT[d, q]  = sum_k Vaug[k, d|ones] PT[k, q]    (K=128 matmul; the extra
                                                  "ones" column makes row 64
                                                  the softmax denominator)
  YT = YuT * recip(denom)  broadcast across partitions via a tiny K=128
  row-selector matmul + reciprocal_approx_fast.

The kernel is one continuous stream: 128 exp ACTIVATEs of [128,1024]
(~142us on ScalarE) paced against the PE.  All projection matmuls are
injected into the PE stream between attention iterations against their
data deadlines.  Inputs load on TWO DMA queues (sync: xe stream;
gpsimd: weights + xd) so the q-side never waits behind the k-side.

Q-columns are HOST-PERMUTED: device column j = 512*qt + i holds logical
s = 4*i + qt, so each qt's 512 columns spread evenly over all 8
output-shard cores (64 each).  This lets the output AllToAll run as 8
small per-(pair,qt) collectives pipelined behind compute; only the last
chunk (~10us) remains on the tail.  The final projection runs in two
qt-halves: half 0 executes during the last AllToAll chunk, half 1
(~8us) after it, accumulating both pairs directly in PSUM.
"""

import sys

sys.path.insert(0, "/opt/trn_rl_repo")

import ml_dtypes
import numpy as np

import bass_rust as _bass_rust

import concourse.bass as bass
import concourse.mybir as mybir
import concourse.tile as tile
from concourse import bacc
from concourse.bass_utils import run_bass_kernel_spmd

F32 = mybir.dt.float32
BF16 = mybir.dt.bfloat16
F8 = mybir.dt.float8e4

# The greedy ACT-table chooser could ping-pong between table sets; hide
# Exp/Ln from the single-function sets so every activation resolves to
# natural_log_exp_and_others.  Only the membership sets are changed — dict
# order/length (the act_func_set_id space) is untouched.
import concourse.hw_specs as _hw_specs
from concourse import bacc as _bacc_mod

_orig_get_tables = _hw_specs.get_activation_tables


def _patched_get_tables(arch):
    t = {k: set(v) for k, v in _orig_get_tables(arch).items()}
    if "natural_log_exp_and_others" in t:
        for name, fns in t.items():
            if name != "natural_log_exp_and_others":
                fns.discard(mybir.ActivationFunctionType.Exp)
                fns.discard(mybir.ActivationFunctionType.Ln)
    return t


_bacc_mod.get_activation_tables = _patched_get_tables

B, S, D, H, HD = 2, 2048, 1024, 16, 64
NCORES = 8
OL = 256          # local output dims (4 heads x 64)
SB = S // 8       # 256: s-slice per core after the 8-way AllToAll
NST = S // 512    # 4 s-tiles of 512
NDC = D // 128    # 8 contraction chunks
NKT = S // 128    # 16 k-tiles
SCALE = 0.125     # 1/sqrt(HD)


def build_nc():
    nc = bacc.Bacc(None, num_devices=NCORES, target_bir_lowering=False)

    xdT = nc.declare_dram_parameter("xdT", [D, S], BF16, isOutput=False)
    xeT = nc.declare_dram_parameter("xeT", [D, S], BF16, isOutput=False)
    wqT = nc.declare_dram_parameter("wqT", [D, OL], BF16, isOutput=False)
    wkT = nc.declare_dram_parameter("wkT", [D, OL], BF16, isOutput=False)
    wvT = nc.declare_dram_parameter("wvT", [D, OL], BF16, isOutput=False)
    wpT = nc.declare_dram_parameter("wpT", [D, D], BF16, isOutput=False)
    # packed biases: rows 0-1 bq, 2-3 bk, 4-11 cb (= Wp@bv + bp)
    bAll = nc.declare_dram_parameter("bAll", [12, 128], F32, isOutput=False)
    ztO = nc.declare_dram_parameter("zT", [2, D, SB], BF16, isOutput=True)

    with tile.TileContext(nc) as tc:
        with (
            tc.tile_pool(name="const", bufs=1) as const,
            tc.tile_pool(name="big", bufs=1) as big,
            tc.tile_pool(name="xp", bufs=1) as xp,
            tc.tile_pool(name="dram", bufs=1, space="DRAM") as dram,
        ):
            # ---- constants / weights resident in SBUF ----
            wq_s = const.tile([128, NDC, OL], BF16)
            wk_s = const.tile([128, NDC, OL], BF16)
            wv_s = const.tile([128, NDC, OL], BF16)
            wp_s = const.tile([128, NDC, D], BF16)
            ball_s = const.tile([128, 12], F32)
            bq_s = ball_s[:, 0:2]
            bk_s = ball_s[:, 2:4]
            cb_s = ball_s[:, 4:12]
            # row-64 selector: out[m, q] = rhs[64, q] via a K=128 matmul
            # (row 64 ones, all other rows zero); db is a persistent,
            # pre-zeroed staging row so the matmul never reads
            # uninitialized SBUF on its zero rows.
            ones_sb = const.tile([128, 128], BF16)
            nc.vector.memset(ones_sb[:], 0.0)
            nc.vector.memset(ones_sb[64:65, :], 1.0)
            db_s = [const.tile([128, 512], BF16, name=f"db{i}") for i in range(2)]
            for i in range(2):
                nc.vector.memset(db_s[i][:], 0.0)

            # persistent activations: head hh of a pair on partitions
            # [64hh, 64hh+64) for both QT and KT2 (enables row-tiled
            # packed scores matmuls, K=64 per head, concurrent).
            QT = [big.tile([128, S], BF16, tag=f"QT{i}", name=f"QT{i}") for i in range(2)]
            KT2 = [big.tile([128, S], BF16, tag=f"KT2{i}", name=f"KT2{i}")
                   for i in range(2)]
            # V augmented with a ones column per head: [k-part, kt, h, 65]
            vaug = big.tile([128, NKT, 4, 65], BF16, tag="vaug")
            nc.vector.memset(vaug[:, :, :, 64:65], 1.0)

            # input staging: one tile + one DMA per 512-s-block.
            # Element [p, dch, two, s] holds d-index dch*256 + two*128 + p,
            # so contraction chunk dc lives at [:, dc // 2, dc % 2, :].
            xe_t = [xp.tile([128, 4, 2, 512], BF16, tag="xe", name=f"xe{st}",
                            bufs=4) for st in range(NST)]
            xd_t = [xp.tile([128, 4, 2, 512], BF16, tag="xd", name=f"xd{qt}",
                            bufs=4) for qt in range(NST)]

            # ---- two-queue input load ----
            # sync queue: the k/v-side stream (xe st0..3).
            # gpsimd queue: weights + biases + q-side (xd) + wp, so the
            # q-side projections never wait behind the xe stream.  Order
            # within the gpsimd queue follows the prologue's consumption:
            # wk (emit_k), wq+xd0+ball (emit_q), wv (emit_v), rest.
            for st in range(NST):
                nc.sync.dma_start(
                    xe_t[st][:],
                    xeT[:, st * 512:(st + 1) * 512]
                    .rearrange("(dch two p) s -> p dch two s", p=128, two=2))
            nc.gpsimd.dma_start(wk_s[:], wkT.rearrange("(dc p) o -> p dc o", p=128))
            nc.gpsimd.dma_start(wq_s[:], wqT.rearrange("(dc p) o -> p dc o", p=128))
            nc.gpsimd.dma_start(
                xd_t[0][:],
                xdT[:, 0:512].rearrange("(dch two p) s -> p dch two s",
                                        p=128, two=2))
            nc.gpsimd.dma_start(ball_s[:], bAll.rearrange("a p -> p a"))
            nc.gpsimd.dma_start(wv_s[:], wvT.rearrange("(dc p) o -> p dc o", p=128))
            for qt in range(1, NST):
                nc.gpsimd.dma_start(
                    xd_t[qt][:],
                    xdT[:, qt * 512:(qt + 1) * 512]
                    .rearrange("(dch two p) s -> p dch two s", p=128, two=2))
            nc.gpsimd.dma_start(wp_s[:], wpT.rearrange("(jc p) o -> p jc o", p=128))

            # per-(pair,qt-half) AllToAll chunks: device q-columns are
            # host-permuted so each qt's 512 columns = 8 dest-cores x 64;
            # two qt's share one 256KB collective (per-op latency ~11us
            # makes smaller chunks inefficient).
            ydramC = [[dram.tile([8, 128, 128], BF16, name=f"ydram{p}_{q}")
                       for q in range(2)] for p in range(2)]
            ygathC = [[dram.tile([8, 128, 128], BF16, name=f"ygath{p}_{q}")
                       for q in range(2)] for p in range(2)]
            # gathered Y^T chunks by half: [j, bb, g, qt%2, q]
            ytgH = [[const.tile([128, 2, 4, 2, 64], BF16, name=f"ytg{p}_{h}")
                     for h in range(2)] for p in range(2)]

            with (
                tc.tile_pool(name="stp", bufs=2, space="PSUM") as stp,
                tc.tile_pool(name="yup", bufs=2, space="PSUM") as yup,
                tc.tile_pool(name="aux", bufs=2, space="PSUM") as auxp,
                tc.tile_pool(name="pt", bufs=6) as ptp,
                tc.tile_pool(name="ep", bufs=6) as ep,
            ):
                # ---- injected projection groups (each uses one aux slot) ----
                def emit_k(st, oc):
                    ssl = slice(st * 512, (st + 1) * 512)
                    kps = auxp.tile([128, 512], F32, tag="aux", name="kps")
                    for dc in range(NDC):
                        nc.tensor.matmul(
                            kps[:], wk_s[:, dc, oc * 128:(oc + 1) * 128],
                            xe_t[st][:, dc // 2, dc % 2, :],
                            start=(dc == 0), stop=(dc == NDC - 1))
                    nc.vector.tensor_scalar_add(
                        KT2[oc][:, ssl], kps[:], bk_s[:, oc:oc + 1])

                def emit_v(st, half):
                    # two s-subblocks (kt = 4*st + 2*half + {0,1}) share one
                    # aux slot; one DVE copy moves both into vaug
                    vps = auxp.tile([128, 2, 256], F32, tag="aux", name="vps")
                    for uu in range(2):
                        u = 2 * half + uu
                        for dc in range(NDC):
                            nc.tensor.matmul(
                                vps[:, uu, :],
                                xe_t[st][:, dc // 2, dc % 2,
                                         u * 128:(u + 1) * 128],
                                wv_s[:, dc, :],
                                start=(dc == 0), stop=(dc == NDC - 1))
                    kt0 = 4 * st + 2 * half
                    nc.vector.tensor_copy(
                        vaug[:, kt0:kt0 + 2, :, 0:64],
                        vps[:].rearrange("p u (h d) -> p u h d", h=4))

                def emit_q(qt, oc):
                    qsl = slice(qt * 512, (qt + 1) * 512)
                    qps = auxp.tile([128, 512], F32, tag="aux", name="qps")
                    for dc in range(NDC):
                        nc.tensor.matmul(
                            qps[:], wq_s[:, dc, oc * 128:(oc + 1) * 128],
                            xd_t[qt][:, dc // 2, dc % 2, :],
                            start=(dc == 0), stop=(dc == NDC - 1))
                    nc.vector.tensor_scalar_add(
                        QT[oc][:, qsl], qps[:], bq_s[:, oc:oc + 1])

                def finish_qt(pair, qt, yufs, anchor):
                    # deferred normalize+store; the raw denominator row is
                    # broadcast across partitions via a K=128 selector
                    # matmul (pinned behind `anchor` so the scheduler
                    # cannot hoist it into a head-of-line block), then the
                    # fast approximate reciprocal runs on the full
                    # [64,512] block at base partition 0.
                    for hh in range(2):
                        nc.vector.tensor_copy(
                            db_s[hh][64:65, :], yufs[hh][64:65, :])
                    rpss = []
                    for hh in range(2):
                        rps = auxp.tile([128, 512], F32, tag="aux", name="rps")
                        rmm = nc.tensor.matmul(
                            rps[:], ones_sb[:, :], db_s[hh][:, :],
                            start=True, stop=True)
                        _bass_rust.add_dep_helper(
                            rmm.ins, anchor.ins, sync=False,
                            reason="pin R-matmul after current attention MMs")
                        rpss.append(rps)
                    ysts = []
                    for hh in range(2):
                        rrec = ep.tile([128, 512], F32, tag="r32", name="rrec")
                        nc.vector.reciprocal_approx_fast(
                            rrec[0:64, :], rpss[hh][0:64, :])
                        yst = ep.tile([64, 512], BF16, tag="yst", name="yst")
                        nc.vector.tensor_mul(
                            yst[:], yufs[hh][0:64, :], rrec[0:64, :])
                        ysts.append(yst)
                    hf, tq = qt // 2, qt % 2
                    for hh in range(2):
                        nc.sync.dma_start(
                            ydramC[pair][hf][:, 64 * hh:64 * (hh + 1),
                                             64 * tq:64 * (tq + 1)]
                            .rearrange("d j q -> j d q"),
                            ysts[hh][:].rearrange("j (d q) -> j d q", d=8))
                    if tq == 1:
                        # pipelined per-half AllToAll + gather (gpsimd queue)
                        nc.gpsimd.collective_compute(
                            "AllToAll", mybir.AluOpType.bypass,
                            replica_groups=[list(range(NCORES))],
                            ins=[ydramC[pair][hf].opt()],
                            outs=[ygathC[pair][hf].opt()])
                        nc.gpsimd.dma_start(
                            ytgH[pair][hf][:],
                            ygathC[pair][hf].rearrange(
                                "(bb g) j (t q) -> j bb g t q", bb=2, t=2))

                # ---- prologue: minimum work before the exp stream starts ----
                emit_k(0, 0)
                emit_q(0, 0)

                # injection schedule: (pair, qt, kt) -> list of thunks.
                # Deadlines: pair0/qt0 consumes KT2[0] st_j at kt=4j and
                # vaug at kt; QT[0] qt at pair0/qt start; KT2[1]/QT[1] only
                # at pair1 (iteration 64+), so their projections ride
                # pair-0's PE slack.
                inj = {}

                def at(pair, qt, kt, fn, *a):
                    inj.setdefault((pair, qt, kt), []).append((fn, a))

                at(0, 0, 0, emit_v, 0, 1)
                at(0, 0, 1, emit_k, 1, 0)
                at(0, 0, 2, emit_v, 1, 0)
                at(0, 0, 3, emit_v, 1, 1)
                at(0, 0, 5, emit_k, 2, 0)
                at(0, 0, 6, emit_v, 2, 0)
                at(0, 0, 7, emit_v, 2, 1)
                at(0, 0, 9, emit_k, 3, 0)
                at(0, 0, 10, emit_v, 3, 0)
                at(0, 0, 11, emit_v, 3, 1)
                at(0, 0, 13, emit_q, 1, 0)
                at(0, 1, 2, emit_k, 0, 1)
                at(0, 1, 6, emit_k, 1, 1)
                at(0, 1, 10, emit_q, 2, 0)
                at(0, 2, 2, emit_k, 2, 1)
                at(0, 2, 10, emit_q, 3, 0)
                at(0, 3, 2, emit_k, 3, 1)
                at(0, 3, 10, emit_q, 0, 1)
                at(1, 0, 10, emit_q, 1, 1)
                at(1, 1, 2, emit_q, 2, 1)
                at(1, 2, 2, emit_q, 3, 1)

                # ---- the attention stream ----
                # Software-pipelined emission: the scores matmuls for
                # iteration n+2 are emitted during iteration n, so they
                # execute inside exp(n)'s window and exp(n+1) is never
                # gated on a fresh scores matmul.  The two heads' scores
                # are row-tiled (K=64, tile_position (0,0)/(64,0)) and run
                # concurrently in one N=512 slot.
                def emit_scores(pair, qt, kt):
                    sps = stp.tile([128, 1024], F32, tag="st")
                    for hh in range(2):
                        psl = slice(64 * hh, 64 * (hh + 1))
                        nc.tensor.matmul(
                            sps[:, 512 * hh:512 * (hh + 1)],
                            KT2[pair][psl, kt * 128:(kt + 1) * 128],
                            QT[pair][psl, qt * 512:(qt + 1) * 512],
                            start=True, stop=True)
                    return sps

                iters = [(pair, qt, kt) for pair in range(2)
                         for qt in range(NST) for kt in range(NKT)]
                pending = None
                yu = None
                # prime the first two scores BEFORE emit_v so exp(0) starts
                # while the first V projection is still on the PE
                sps_q = [emit_scores(*iters[0]), emit_scores(*iters[1])]
                emit_v(0, 0)
                for idx, (pair, qt, kt) in enumerate(iters):
                    if kt == 0:
                        yu = [yup.tile([128, 512], F32, tag="yu",
                                       name=f"yu{hh}") for hh in range(2)]
                    sps = sps_q.pop(0)
                    pt_t = ptp.tile([128, 1024], BF16, tag="pt")
                    nc.scalar.activation(
                        pt_t[:], sps[:],
                        mybir.ActivationFunctionType.Exp, scale=SCALE)
                    if idx + 2 < len(iters):
                        sps_q.append(emit_scores(*iters[idx + 2]))
                    for hh in range(2):
                        h = 2 * pair + hh
                        last_pv = nc.tensor.matmul(
                            yu[hh][0:65, :], vaug[:, kt, h, :],
                            pt_t[:, 512 * hh:512 * (hh + 1)],
                            start=(kt == 0), stop=(kt == NKT - 1))
                    for fn, a in inj.get((pair, qt, kt), ()):
                        fn(*a)
                    if kt == 6 and pending is not None:
                        finish_qt(pair, *pending, anchor=last_pv)
                        pending = None
                    if kt == NKT - 1:
                        # evacuate PSUM immediately (DVE only, no PE ops)
                        yufs = []
                        for hh in range(2):
                            yuf = ep.tile([65, 512], F32, tag="yuf",
                                          name="yuf")
                            nc.vector.tensor_copy(yuf[:], yu[hh][0:65, :])
                            yufs.append(yuf)
                        pending = (qt, yufs)
                        if qt == NST - 1:
                            finish_qt(pair, *pending, anchor=last_pv)
                            pending = None

            # ---- output projection, two qt-halves: half 0 runs during the
            # last AllToAll chunk, half 1 right after it.  Both pairs
            # accumulate directly in PSUM (8 matmuls per oc, N=256). ----
            with (
                tc.tile_pool(name="zps", bufs=4, space="PSUM") as zpsp,
                tc.tile_pool(name="zt", bufs=4) as ztp,
            ):
                for hf in range(2):
                    for oc in range(NDC):
                        zps = zpsp.tile([128, 256], F32, tag="z", name="zps")
                        n = 0
                        for pair in range(2):
                            for g in range(4):
                                nc.tensor.matmul(
                                    zps[:],
                                    wp_s[:, 2 * g + pair,
                                         oc * 128:(oc + 1) * 128],
                                    ytgH[pair][hf][:, :, g, :, :],
                                    start=(n == 0), stop=(n == 7))
                                n += 1
                        zt_t = ztp.tile([128, 256], BF16, tag="zt", name="zt_t")
                        nc.vector.tensor_scalar_add(
                            zt_t[:], zps[:], cb_s[:, oc:oc + 1])
                        nc.sync.dma_start(
                            ztO[:, oc * 128:(oc + 1) * 128,
                                128 * hf:128 * (hf + 1)]
                            .rearrange("b p q -> p b q"),
                            zt_t[:].rearrange("p (b q) -> p b q", b=2))

    nc.compile()
    return nc


# device q-column j = 512*qt + i holds logical s = 4*i + qt
_QPERM = (4 * np.arange(512)[None, :] + np.arange(4)[:, None]).reshape(-1)
# core-local output column 64*qt + il holds logical s-offset 4*il + qt
_OPERM = (4 * np.arange(64)[None, :] + np.arange(4)[:, None]).reshape(-1)


def make_in_maps(decoder_hs, encoder_hs, Wq, bq, Wk, bk, Wv, bv, Wp, bp):
    dh = np.ascontiguousarray(np.asarray(decoder_hs, np.float32))
    eh = np.ascontiguousarray(np.asarray(encoder_hs, np.float32))
    Wq, Wk, Wv, Wp = (np.asarray(a, np.float32) for a in (Wq, Wk, Wv, Wp))
    bq, bk, bv, bp = (np.asarray(a, np.float32) for a in (bq, bk, bv, bp))
    c = (Wp @ bv + bp).astype(np.float32)
    bf = ml_dtypes.bfloat16
    wpT = np.ascontiguousarray(Wp.T).astype(bf)
    xdT = [np.ascontiguousarray(dh[b].T[:, _QPERM]).astype(bf) for b in range(B)]
    xeT = [np.ascontiguousarray(eh[b].T).astype(bf) for b in range(B)]
    in_maps = []
    for core in range(NCORES):
        b, r = divmod(core, 4)
        sl = slice(OL * r, OL * (r + 1))
        ball = np.concatenate(
            [bq[sl].reshape(2, 128), bk[sl].reshape(2, 128),
             c.reshape(8, 128)], axis=0)
        in_maps.append({
            "xdT": xdT[b],
            "xeT": xeT[b],
            "wqT": np.ascontiguousarray(Wq[sl].T).astype(bf),
            "wkT": np.ascontiguousarray(Wk[sl].T).astype(bf),
            "wvT": np.ascontiguousarray(Wv[sl].T).astype(bf),
            "wpT": wpT,
            "bAll": np.ascontiguousarray(ball),
        })
    return in_maps


def assemble_output(results):
    out = np.empty((B, S, D), np.float32)
    for core in range(NCORES):
        zT = np.asarray(results[core]["zT"])  # [2, 1024, 256]
        for b in range(B):
            out[b, SB * core + _OPERM, :] = zT[b].T
    return out


_NC = None


def kernel(**inputs):
    global _NC
    if _NC is None:
        _NC = build_nc()
    in_maps = make_in_maps(**inputs)
    res = run_bass_kernel_spmd(_NC, in_maps, list(range(NCORES)))
    return assemble_output(res.results)


if __name__ == "__main__":
    nc = build_nc()
    print("built ok")
